# revision 1
# baseline (speedup 1.0000x reference)
"""BitnetMLP on 8 TRN2 NeuronCores — Megatron tensor-parallel over the
intermediate dim I, with exact integer arithmetic on the TensorEngine.

Math: activation fake-quant makes activations exact int8 values and weight
fake-quant makes weights exact ternary values. Both are exactly representable
in bf16/fp8e4, and PSUM accumulates in f32, so every matmul is computed as an
exact integer matmul at full bf16 speed; per-token / per-tensor dequant scales
are applied to the f32 partial sums afterward.

Sharding (per core r of 8):
  w_gate/w_up: I-column shard (1024 of 8192)  -> h^T shard [I_sh=1024, T]
  w_down:      I-row shard                    -> partial y, ReduceScatter(add)
  per-token RMS var and abs-max stats over the full I: AllReduce add / max.

Layouts are feature-major (host pre-transposes x and the weights so the
contract dim lands on SBUF partitions; no on-device transposes of x/w/h).

Structure: an x-quant prepass streams exact-int bf16 x^T tiles to DRAM so the
main per-group matmul pipeline has no latency chains (DRAM gathers / AllReduce
waits overlap matmuls of neighboring groups).
"""
import numpy as np

N_CORES = 8
B, S, H, I = 2, 2048, 2048, 8192
T = B * S                      # 4096 tokens
ISH = I // N_CORES             # 1024  I shard per core
TG = 512                       # tokens per group
NG = T // TG                   # 8 groups
KC = H // 128                  # 16 contract chunks for gate/up
IC = ISH // 128                # 8  contract chunks for down / h^T partition chunks
NH = 2048 // 512               # 4  output col groups for down
NTC = TG // 128                # 4  token tiles per group
RS_BATCH = 1                   # groups per ReduceScatter
NB = NG // RS_BATCH            # 4 RS batches

MAGIC = float(1.5 * 2 ** 23)   # f32 round-to-nearest-even forcing constant
EPS = 1e-5
RMS_EPS = 1e-6

_CACHED = {}


def _build():
    import concourse.bass as bass
    import concourse.bacc as bacc
    import concourse.tile as tile
    import concourse.mybir as mybir
    from concourse import masks
    from contextlib import ExitStack

    dt = mybir.dt
    AO = mybir.AluOpType
    AF = mybir.ActivationFunctionType
    RG = [list(range(N_CORES))]

    nc = bacc.Bacc("TRN2", target_bir_lowering=False, debug=False,
                   num_devices=N_CORES)

    xT_in = nc.dram_tensor("xT", [H, T], dt.float32, kind="ExternalInput")
    wgT_in = nc.dram_tensor("wgT", [H, ISH], dt.float32, kind="ExternalInput")
    wuT_in = nc.dram_tensor("wuT", [H, ISH], dt.float32, kind="ExternalInput")
    wdT_in = nc.dram_tensor("wdT", [ISH, 2048], dt.float32, kind="ExternalInput")
    lnw_in = nc.dram_tensor("lnw", [ISH], dt.float32, kind="ExternalInput")
    y_out = nc.dram_tensor("y_out", [T // N_CORES, 2048], dt.float32,
                           kind="ExternalOutput")

    with tile.TileContext(nc) as tc:
        with ExitStack() as stack:
            ep = stack.enter_context
            constp = ep(tc.tile_pool(name="const", bufs=1))
            wqp = ep(tc.tile_pool(name="wq", bufs=1))
            wstage = ep(tc.tile_pool(name="wstage", bufs=4))
            xstage = ep(tc.tile_pool(name="xstage", bufs=2))
            qxp = ep(tc.tile_pool(name="qx", bufs=2))
            hbp = ep(tc.tile_pool(name="hbuf", bufs=2))
            qhp = ep(tc.tile_pool(name="qh", bufs=2))
            bcp = ep(tc.tile_pool(name="bc", bufs=2))
            sxp = ep(tc.tile_pool(name="sxal", bufs=1))
            rtp = ep(tc.tile_pool(name="rt", bufs=3))
            yrp = ep(tc.tile_pool(name="yrow", bufs=1))
            smp = ep(tc.tile_pool(name="small", bufs=2))
            rowp = ep(tc.tile_pool(name="rows", bufs=2))
            rowp2 = ep(tc.tile_pool(name="rows2", bufs=1))
            evp = ep(tc.tile_pool(name="evac", bufs=2))
            ps_gu = ep(tc.tile_pool(name="ps_gu", bufs=3, space="PSUM"))
            ps_dn = ep(tc.tile_pool(name="ps_dn", bufs=2, space="PSUM"))
            ps_ss = ep(tc.tile_pool(name="ps_ss", bufs=1, space="PSUM"))
            ps_misc = ep(tc.tile_pool(name="ps_misc", bufs=2, space="PSUM"))
            dram = ep(tc.tile_pool(name="dram", bufs=1, space="DRAM"))
            dram_rs = ep(tc.tile_pool(name="dram_rs", bufs=8, space="DRAM"))

            # ---------- constants ----------
            ident = constp.tile([128, 128], dt.float32)
            masks.make_identity(nc, ident[:])
            ones_col = constp.tile([128, 1], dt.float32)   # lhsT for partition sums
            nc.vector.memset(ones_col[:], 1.0)
            ones_col_bf = constp.tile([128, 1], dt.bfloat16)
            nc.vector.memset(ones_col_bf[:], 1.0)
            ones_row = constp.tile([1, 128], dt.float32)   # lhsT for K=1 broadcasts
            nc.vector.memset(ones_row[:], 1.0)
            lnw_sb = constp.tile([128, IC], dt.float32)    # lnw[128*ic + p] at [p, ic]
            nc.sync.dma_start(lnw_sb[:], lnw_in.rearrange("(c p) -> p c", p=128)[:])
            alnw_sb = constp.tile([128, IC], dt.float32)   # |lnw|
            nc.vector.tensor_scalar(alnw_sb.bitcast(dt.uint32)[:],
                                    lnw_sb.bitcast(dt.uint32)[:],
                                    0x7FFFFFFF, None, AO.bitwise_and)

            # ---------- internal DRAM ----------
            y_partial = dram.tile([T, 2048], dt.bfloat16)
            stat_in = dram.tile([NG, 2, TG], dt.float32)
            stat_out = dram.tile([NG, 2 * N_CORES, TG], dt.float32)
            wsum_part = dram.tile([8], dt.float32)
            wsum_glob = dram.tile([8], dt.float32)
            row_bounce = dram.tile([NG, 5, TG], dt.float32)  # sx / cg+cu / al / spare

            # ---------- weight abs-sum stats ----------
            wsum_row = rowp.tile([1, 8], dt.float32, tag="wsum_row")
            for wi, (w_in, nchunk, wcols) in enumerate((
                    (wgT_in, KC, ISH), (wuT_in, KC, ISH), (wdT_in, IC, 2048))):
                acc = smp.tile([128, 1], dt.float32, tag="wacc")
                for c in range(nchunk):
                    for cc in range(wcols // 1024):
                        st = wstage.tile([128, 1024], dt.float32, tag="wstage")
                        nc.sync.dma_start(st[:], w_in[c * 128:(c + 1) * 128,
                                                      cc * 1024:(cc + 1) * 1024])
                        red = smp.tile([128, 1], dt.float32, tag="wred")
                        nc.vector.tensor_reduce(red[:], st[:], mybir.AxisListType.X,
                                                AO.add, apply_absolute_value=True)
                        if c == 0 and cc == 0:
                            nc.vector.tensor_copy(acc[:], red[:])
                        else:
                            nc.vector.tensor_tensor(acc[:], acc[:], red[:], AO.add)
                wsum_ps = ps_misc.tile([128, 512], dt.float32, tag="misc_ps")
                nc.tensor.matmul(wsum_ps[0:1, 0:1], ones_col[:], acc[:], start=True,
                                 stop=True)
                nc.scalar.copy(wsum_row[:, wi:wi + 1], wsum_ps[0:1, 0:1])
            nc.vector.memset(wsum_row[:, 3:8], 0.0)
            nc.sync.dma_start(wsum_part.rearrange("(o f) -> o f", o=1)[:], wsum_row[:])
            nc.gpsimd.collective_compute(
                "AllReduce", AO.add, replica_groups=RG,
                ins=[wsum_part.opt()], outs=[wsum_glob.opt()])

            # scl_row: [sw_g, sw_u, sw_d, mg/127, mu/127, md, 0, 0]
            wsg_row = rowp.tile([1, 8], dt.float32, tag="wsg_row")
            nc.sync.dma_start(wsg_row[:], wsum_glob.rearrange("(o f) -> o f", o=1)[:])
            mean_row = rowp.tile([1, 8], dt.float32, tag="mean_row")
            nc.vector.tensor_scalar(mean_row[:, 0:3], wsg_row[:, 0:3],
                                    float(1.0 / (I * H)), EPS, AO.mult, AO.max)
            scl_row = rowp.tile([1, 8], dt.float32, tag="scl_row")
            rw = rowp.tile([1, 8], dt.float32, tag="rw_row")
            nc.vector.reciprocal(rw[:, 0:3], mean_row[:, 0:3])
            nt = rowp.tile([1, 8], dt.float32, tag="nt_row")
            nc.vector.tensor_tensor(nt[:, 0:3], mean_row[:, 0:3], rw[:, 0:3], AO.mult)
            nc.vector.tensor_scalar(nt[:, 0:3], nt[:, 0:3], -1.0, 2.0, AO.mult, AO.add)
            nc.vector.tensor_tensor(scl_row[:, 0:3], rw[:, 0:3], nt[:, 0:3], AO.mult)
            nc.vector.tensor_copy(scl_row[:, 3:5], mean_row[:, 0:2])
            nc.vector.tensor_copy(scl_row[:, 5:6], mean_row[:, 2:3])
            nc.vector.memset(scl_row[:, 6:8], 0.0)
            wst_ps = ps_misc.tile([128, 512], dt.float32, tag="misc_ps")
            nc.tensor.matmul(wst_ps[:, 0:8], ones_row[:], scl_row[:], start=True,
                             stop=True)
            wstats = constp.tile([128, 8], dt.float32)
            nc.vector.tensor_copy(wstats[:], wst_ps[:, 0:8])

            # ---------- quantize weights to ternary fp8 ----------
            qwg = wqp.tile([128, KC * ISH], dt.float8e4)
            qwu = wqp.tile([128, KC * ISH], dt.float8e4)
            qwd = wqp.tile([128, IC * 2048], dt.float8e4)
            for (w_in, qw, nchunk, wcols, si) in (
                (wgT_in, qwg, KC, ISH, 0), (wuT_in, qwu, KC, ISH, 1),
                (wdT_in, qwd, IC, 2048, 2),
            ):
                for c in range(nchunk):
                    for cc in range(wcols // 1024):
                        st = wstage.tile([128, 1024], dt.float32, tag="wstage")
                        nc.sync.dma_start(st[:], w_in[c * 128:(c + 1) * 128,
                                                      cc * 1024:(cc + 1) * 1024])
                        nc.vector.tensor_scalar(st[:], st[:], wstats[:, si:si + 1],
                                                MAGIC, AO.mult, AO.add)
                        nc.vector.tensor_scalar(st[:], st[:], -MAGIC, 1.0, AO.add,
                                                AO.min)
                        o0 = c * wcols + cc * 1024
                        nc.vector.tensor_scalar(qw[:, o0:o0 + 1024], st[:],
                                                -1.0, None, AO.max)

            # ---------- x-quant prepass (emitted interleaved, fills qxT slots) --
            qxT_slots = {}

            def emit_prepass(g):
                tok0 = g * TG
                xmax = smp.tile([128, TG], dt.float32, tag="xmax")
                for kc in range(KC):
                    st = xstage.tile([128, TG], dt.float32, tag="xs")
                    nc.sync.dma_start(st[:], xT_in[kc * 128:(kc + 1) * 128,
                                                   tok0:tok0 + TG])
                    if kc == 0:
                        nc.scalar.activation(xmax[:], st[:], AF.Abs)
                    else:
                        nc.scalar.activation(st[:], st[:], AF.Abs)
                        nc.vector.tensor_tensor(xmax[:], xmax[:], st[:], AO.max)
                mx_nat = smp.tile([128, NTC], dt.float32, tag="mx_nat")
                for c in range(NTC):
                    tr_ps = ps_misc.tile([128, 512], dt.float32, tag="misc_ps")
                    nc.tensor.transpose(tr_ps[:, 0:128],
                                        xmax[:, c * 128:(c + 1) * 128], ident[:])
                    nc.vector.tensor_reduce(mx_nat[:, c:c + 1], tr_ps[:, 0:128],
                                            mybir.AxisListType.X, AO.max)
                nc.vector.tensor_scalar(mx_nat[:], mx_nat[:], EPS, None, AO.max)
                # sx = 127/mxc (reciprocal + newton)
                r0 = smp.tile([128, NTC], dt.float32, tag="sx_r0")
                nc.vector.reciprocal(r0[:], mx_nat[:])
                ntr = smp.tile([128, NTC], dt.float32, tag="sx_nt")
                nc.vector.tensor_tensor(ntr[:], mx_nat[:], r0[:], AO.mult)
                nc.vector.tensor_scalar(ntr[:], ntr[:], -1.0, 2.0, AO.mult, AO.add)
                sxn = smp.tile([128, NTC], dt.float32, tag="sxn")
                nc.vector.tensor_tensor(sxn[:], r0[:], ntr[:], AO.mult)
                nc.vector.tensor_scalar(sxn[:], sxn[:], 127.0, None, AO.mult)
                # mc = mxc/127 split into exact power-of-two mp and residual r
                sxmc = smp.tile([128, 3 * NTC], dt.float32, tag="sxmc")
                nc.vector.tensor_copy(sxmc[:, 0:NTC], sxn[:])
                mc_nat = smp.tile([128, NTC], dt.float32, tag="mc_nat")
                nc.vector.tensor_scalar(mc_nat[:], mx_nat[:],
                                        float(1.0 / 127.0), None, AO.mult)
                nc.vector.tensor_scalar(
                    sxmc.bitcast(dt.uint32)[:, NTC:2 * NTC],
                    mc_nat.bitcast(dt.uint32)[:], 0x7F800000, None, AO.bitwise_and)
                nc.vector.tensor_scalar(
                    sxmc.bitcast(dt.uint32)[:, 2 * NTC:3 * NTC],
                    mc_nat.bitcast(dt.uint32)[:], 0x007FFFFF, 0x3F800000,
                    AO.bitwise_and, AO.bitwise_or)
                nc.sync.dma_start(
                    row_bounce[g, 0:3].rearrange("s (c p) -> p s c", p=128)[:],
                    sxmc.rearrange("p (s c) -> p s c", c=NTC)[:])
                sx_tile = sxp.tile([128, TG], dt.float32, tag="sx_tile")
                nc.sync.dma_start(sx_tile[:], row_bounce[g, 0]
                                  .rearrange("(o f) -> o f", o=1)
                                  .partition_broadcast(128))
                mc_tile = sxp.tile([128, TG], dt.float32, tag="mc_tile")
                nc.sync.dma_start(mc_tile[:], row_bounce[g, 1]
                                  .rearrange("(o f) -> o f", o=1)
                                  .partition_broadcast(128))
                r_tile = rtp.tile([128, TG], dt.float32, tag="r_tile")
                rt_slots[g] = r_tile
                nc.sync.dma_start(r_tile[:], row_bounce[g, 2]
                                  .rearrange("(o f) -> o f", o=1)
                                  .partition_broadcast(128))
                qxT = qxp.tile([128, KC * TG], dt.bfloat16, tag="qxT")
                qxT_slots[g] = qxT
                for kc in range(KC):
                    st = xstage.tile([128, TG], dt.float32, tag="xs")
                    nc.sync.dma_start(st[:], xT_in[kc * 128:(kc + 1) * 128,
                                                   tok0:tok0 + TG])
                    nc.vector.tensor_tensor(st[:], st[:], sx_tile[:], AO.mult)
                    nc.vector.tensor_scalar(st[:], st[:], MAGIC, -MAGIC, AO.add,
                                            AO.add)
                    nc.vector.tensor_scalar(st[:], st[:], 127.0, -128.0, AO.min,
                                            AO.max)
                    nc.vector.tensor_tensor(qxT[:, kc * TG:(kc + 1) * TG], st[:],
                                            mc_tile[:], AO.mult)

            # ---------- main pipeline (software-pipelined emission) ----------
            cd_slots = {}
            hT_slots = {}
            rs_slots = {}
            al_slots = {}
            qh_slots = {}
            r_slots = {}
            rt_slots = {}

            def emit_phase1(g):
                tok0 = g * TG
                qxT = qxT_slots.pop(g)
                r_tile = rt_slots.pop(g)
                hT = hbp.tile([128, IC * TG], dt.float32, tag="hT")
                hT_slots[g] = hT
                maxt = smp.tile([128, TG], dt.float32, tag="maxt")
                ss_ps = ps_ss.tile([1, TG], dt.float32, tag="ss_ps")
                h2_prev = [None]

                def emit_ss(ic_done):
                    nc.tensor.matmul(ss_ps[:], ones_col_bf[:], h2_prev[0][:],
                                     start=(ic_done == 0), stop=(ic_done == IC - 1))

                for ic in range(IC):
                    g_ps = ps_gu.tile([128, TG], dt.float32, tag="gu_ps")
                    u_ps = ps_gu.tile([128, TG], dt.float32, tag="gu_ps")
                    for kc in range(KC):
                        nc.tensor.matmul(
                            g_ps[:],
                            qwg[:, kc * ISH + ic * 128: kc * ISH + (ic + 1) * 128],
                            qxT[:, kc * TG:(kc + 1) * TG],
                            start=(kc == 0), stop=(kc == KC - 1))
                    for kc in range(KC):
                        nc.tensor.matmul(
                            u_ps[:],
                            qwu[:, kc * ISH + ic * 128: kc * ISH + (ic + 1) * 128],
                            qxT[:, kc * TG:(kc + 1) * TG],
                            start=(kc == 0), stop=(kc == KC - 1))
                    if ic > 0:
                        emit_ss(ic - 1)
                    gv = evp.tile([128, TG], dt.float32, tag="gsv")
                    nc.vector.tensor_tensor(gv[:], g_ps[:], r_tile[:], AO.mult)
                    sv = evp.tile([128, TG], dt.float32, tag="gsv")
                    nc.scalar.activation(sv[:], gv[:], AF.Silu,
                                         scale=wstats[:, 3:4])
                    hslice = hT[:, ic * TG:(ic + 1) * TG]
                    nc.vector.scalar_tensor_tensor(hslice, u_ps[:],
                                                   wstats[:, 4:5], sv[:],
                                                   AO.mult, AO.mult)
                    h2 = evp.tile([128, TG], dt.bfloat16, tag="h2")
                    nc.vector.tensor_tensor(h2[:], hslice, hslice, AO.mult)
                    h2_prev[0] = h2
                    ha = evp.tile([128, TG], dt.float32, tag="ha")
                    nc.vector.tensor_scalar(ha.bitcast(dt.uint32)[:],
                                            hT.bitcast(dt.uint32)[:, ic * TG:(ic + 1) * TG],
                                            0x7FFFFFFF, None, AO.bitwise_and)
                    if ic == 0:
                        nc.vector.tensor_scalar(maxt[:], ha[:],
                                                alnw_sb[:, 0:1], None, AO.mult)
                    else:
                        nc.vector.scalar_tensor_tensor(maxt[:], ha[:],
                                                       alnw_sb[:, ic:ic + 1], maxt[:],
                                                       AO.mult, AO.max)
                emit_ss(IC - 1)
                pm_nat = smp.tile([128, NTC], dt.float32, tag="pm_nat")
                for c in range(NTC):
                    tr_ps = ps_misc.tile([128, 512], dt.float32, tag="misc_ps")
                    nc.tensor.transpose(tr_ps[:, 0:128],
                                        maxt[:, c * 128:(c + 1) * 128], ident[:])
                    nc.vector.tensor_reduce(pm_nat[:, c:c + 1], tr_ps[:, 0:128],
                                            mybir.AxisListType.X, AO.max)
                ss_row = rowp.tile([1, TG], dt.float32, tag="grow")
                nc.vector.tensor_copy(ss_row[:], ss_ps[:])
                nc.gpsimd.dma_start(stat_in[g, 0].rearrange("(o f) -> o f", o=1)[:],
                                    ss_row[:])
                nc.gpsimd.dma_start(stat_in[g, 1].rearrange("(c p) -> p c", p=128)[:],
                                    pm_nat[:])
                nc.gpsimd.collective_compute(
                    "AllGather", AO.bypass, replica_groups=RG,
                    ins=[stat_in[g].opt()], outs=[stat_out[g].opt()])

            def emit_phase2a(g):
                tok0 = g * TG
                J = TG // 32
                # gathered stats [16, TG] -> [32, TG] tile; rows 16:32 zeroed
                stat32 = smp.tile([32, TG], dt.float32, tag="stat32")
                nc.vector.memset(stat32[:], 0.0)
                nc.gpsimd.dma_start(stat32[0:2 * N_CORES, :], stat_out[g])
                st32 = smp.tile([32, TG], dt.float32, tag="st32")
                nc.vector.transpose(st32[:], stat32[:])
                # st32[q, 32j + 16h + 2a + kind]: token t=32j+q, rank a, h=1 junk
                stv = st32.rearrange("p (j h a two) -> p j h two a",
                                     h=2, two=2, a=N_CORES)
                ssg = smp.tile([32, J], dt.float32, tag="ssg")
                nc.vector.tensor_reduce(ssg[:], stv[:, :, 0:1, 0:1, :],
                                        mybir.AxisListType.X, AO.add)
                pmg = smp.tile([32, J], dt.float32, tag="pmg")
                nc.vector.tensor_reduce(pmg[:], stv[:, :, 0:1, 1:2, :],
                                        mybir.AxisListType.X, AO.max)
                # r residual in [32, J] layout (t = 32j + q)
                r32 = smp.tile([32, J], dt.float32, tag="r32")
                nc.sync.dma_start(r32[:], row_bounce[g, 2]
                                  .rearrange("(j q) -> q j", q=32)[:])
                nc.vector.tensor_tensor(pmg[:], pmg[:], r32[:], AO.mult)
                rr2 = smp.tile([32, J], dt.float32, tag="rr2")
                nc.vector.tensor_tensor(rr2[:], r32[:], r32[:], AO.mult)
                nc.vector.tensor_tensor(ssg[:], ssg[:], rr2[:], AO.mult)
                vr = smp.tile([32, J], dt.float32, tag="vr")
                nc.vector.tensor_scalar(vr[:], ssg[:], float(1.0 / I), RMS_EPS,
                                        AO.mult, AO.add)
                sq = smp.tile([32, J], dt.float32, tag="sq")
                nc.scalar.sqrt(sq[:], vr[:])
                rr = smp.tile([32, J], dt.float32, tag="rr")
                nc.vector.reciprocal(rr[:], sq[:])
                ntn = smp.tile([32, J], dt.float32, tag="ntn")
                nc.vector.tensor_tensor(ntn[:], sq[:], rr[:], AO.mult)
                nc.vector.tensor_scalar(ntn[:], ntn[:], -1.0, 2.0, AO.mult, AO.add)
                nc.vector.tensor_tensor(rr[:], rr[:], ntn[:], AO.mult)
                rmc = smp.tile([32, J], dt.float32, tag="rmc")
                nc.vector.tensor_tensor(rmc[:], rr[:], pmg[:], AO.mult)
                nc.vector.tensor_scalar(rmc[:], rmc[:], EPS, None, AO.max)
                cd32 = smp.tile([32, J], dt.float32, tag="cd32")
                nc.vector.tensor_scalar(cd32[:], rmc[:], wstats[0:32, 5:6],
                                        float(1.0 / 127.0), AO.mult, AO.mult)
                nc.sync.dma_start(row_bounce[g, 4]
                                  .rearrange("(j q) -> q j", q=32)[:], cd32[:])
                cd = smp.tile([128, NTC], dt.float32, tag="cd")
                cd_slots[g] = cd
                nc.sync.dma_start(cd[:], row_bounce[g, 4]
                                  .rearrange("(c p) -> p c", p=128)[:])
                ar0 = smp.tile([32, J], dt.float32, tag="ar0")
                nc.vector.reciprocal(ar0[:], rmc[:])
                ntn2 = smp.tile([32, J], dt.float32, tag="ntn2")
                nc.vector.tensor_tensor(ntn2[:], rmc[:], ar0[:], AO.mult)
                nc.vector.tensor_scalar(ntn2[:], ntn2[:], -1.0, 2.0, AO.mult, AO.add)
                nc.vector.tensor_tensor(ar0[:], ar0[:], ntn2[:], AO.mult)
                al32 = smp.tile([32, J], dt.float32, tag="al32")
                nc.vector.tensor_tensor(al32[:], rr[:], ar0[:], AO.mult)
                nc.vector.tensor_scalar(al32[:], al32[:], 127.0, None, AO.mult)
                nc.vector.tensor_tensor(al32[:], al32[:], r32[:], AO.mult)
                nc.sync.dma_start(row_bounce[g, 3]
                                  .rearrange("(j q) -> q j", q=32)[:], al32[:])
                al_tile = sxp.tile([128, TG], dt.float32, tag="al_tile")
                al_slots[g] = al_tile
                nc.sync.dma_start(al_tile[:], row_bounce[g, 3]
                                  .rearrange("(o f) -> o f", o=1)
                                  .partition_broadcast(128))

            def emit_phase2q(g):
                tok0 = g * TG
                hT = hT_slots.pop(g)
                al_tile = al_slots.pop(g)
                # quantize h
                qhT = qhp.tile([128, IC * TG], dt.bfloat16, tag="qhT")
                qh_slots[g] = qhT
                for ic in range(IC):
                    tq = evp.tile([128, TG], dt.float32, tag="hq_t")
                    nc.vector.scalar_tensor_tensor(tq[:], hT[:, ic * TG:(ic + 1) * TG],
                                                   lnw_sb[:, ic:ic + 1], al_tile[:],
                                                   AO.mult, AO.mult)
                    nc.vector.tensor_scalar(tq[:], tq[:], MAGIC, -MAGIC, AO.add,
                                            AO.add)
                    nc.vector.tensor_scalar(qhT[:, ic * TG:(ic + 1) * TG], tq[:],
                                            127.0, -128.0, AO.min, AO.max)

            def emit_phase2d(g):
                tok0 = g * TG
                qhT = qh_slots.pop(g)
                # down matmuls + dequant + wide store
                cd = cd_slots.pop(g)
                for tcx in range(NTC):
                    y_row = yrp.tile([128, 2048], dt.bfloat16, tag="y_row")
                    for nh in range(NH):
                        y_ps = ps_dn.tile([128, 512], dt.float32, tag="y_ps")
                        for ic in range(IC):
                            nc.tensor.matmul(
                                y_ps[:],
                                qhT[:, ic * TG + tcx * 128: ic * TG + (tcx + 1) * 128],
                                qwd[:, ic * 2048 + nh * 512: ic * 2048 + (nh + 1) * 512],
                                start=(ic == 0), stop=(ic == IC - 1))
                        nc.scalar.mul(y_row[:, nh * 512:(nh + 1) * 512], y_ps[:],
                                      cd[:, tcx:tcx + 1])
                    nc.sync.dma_start(
                        y_partial[tok0 + tcx * 128: tok0 + (tcx + 1) * 128, :],
                        y_row[:])

                # per-group reduce-scatter; output copy deferred to the tail
                rs_out = dram_rs.tile([TG // N_CORES, 2048], dt.bfloat16,
                                      tag="rs_out")
                rs_slots[g] = rs_out
                nc.gpsimd.collective_compute(
                    "ReduceScatter", AO.add, replica_groups=RG,
                    ins=[y_partial[tok0:tok0 + TG, :].opt()],
                    outs=[rs_out.opt()])

            # interleaved emission: PE stream stays dense across AR latency
            rpb = TG // N_CORES

            def emit_ycast(g):
                yb = rowp2.tile([rpb, 2048], dt.bfloat16, tag="yb")
                nc.sync.dma_start(yb[:], rs_slots.pop(g)[:])
                for cc in range(4):
                    yf = rowp2.tile([rpb, 512], dt.float32, tag="yf")
                    nc.vector.tensor_copy(yf[:], yb[:, cc * 512:(cc + 1) * 512])
                    nc.sync.dma_start(
                        y_out[g * rpb:(g + 1) * rpb, cc * 512:(cc + 1) * 512], yf[:])

            emit_prepass(0)
            emit_prepass(1)
            for g in range(NG):
                emit_phase1(g)
                if g >= 1:
                    emit_phase2a(g - 1)
                    emit_phase2q(g - 1)
                if g + 2 < NG:
                    emit_prepass(g + 2)
                if g >= 2:
                    emit_phase2d(g - 2)
                if g >= 4:
                    emit_ycast(g - 4)
            emit_phase2d(NG - 2)
            emit_phase2a(NG - 1)
            emit_phase2q(NG - 1)
            emit_phase2d(NG - 1)
            for g in range(NG - 4, NG):
                emit_ycast(g)

    nc.compile()
    return nc


def _get_nc():
    if "nc" not in _CACHED:
        _CACHED["nc"] = _build()
    return _CACHED["nc"]


def _make_in_maps(x, w_gate, w_up, w_down, ln_weight):
    xf = np.ascontiguousarray(np.asarray(x, dtype=np.float32).reshape(T, H).T)
    wgT = np.asarray(w_gate, dtype=np.float32).T   # [H, I]
    wuT = np.asarray(w_up, dtype=np.float32).T     # [H, I]
    wdT = np.asarray(w_down, dtype=np.float32).T   # [I, H]
    lnw = np.asarray(ln_weight, dtype=np.float32)
    in_maps = []
    for r in range(N_CORES):
        c0 = r * ISH
        in_maps.append({
            "xT": xf,
            "wgT": np.ascontiguousarray(wgT[:, c0:c0 + ISH]),
            "wuT": np.ascontiguousarray(wuT[:, c0:c0 + ISH]),
            "wdT": np.ascontiguousarray(wdT[c0:c0 + ISH, :]),
            "lnw": np.ascontiguousarray(lnw[c0:c0 + ISH]),
        })
    return in_maps


def _assemble(results):
    out = np.empty((T, 2048), dtype=np.float32)
    rows_per_batch = RS_BATCH * TG // N_CORES          # 128
    for r in range(N_CORES):
        yr = results[r]["y_out"]
        for b in range(NB):
            t0 = b * RS_BATCH * TG + r * rows_per_batch
            out[t0:t0 + rows_per_batch] = \
                yr[b * rows_per_batch:(b + 1) * rows_per_batch]
    return out.reshape(B, S, 2048)


def kernel(x, w_gate, w_up, w_down, ln_weight):
    from concourse import bass_utils

    nc = _get_nc()
    in_maps = _make_in_maps(x, w_gate, w_up, w_down, ln_weight)
    res = bass_utils.run_bass_kernel_spmd(nc, in_maps,
                                          core_ids=list(range(N_CORES)))
    return _assemble(res.results)



# revision 11
# speedup vs baseline: 1.2138x; 1.2138x over previous
"""BitnetMLP on 8 TRN2 NeuronCores — Megatron tensor-parallel over the
intermediate dim I, with exact integer arithmetic on the TensorEngine.

Math: activation fake-quant makes activations exact int8 values and weight
fake-quant makes weights exact ternary values. Ints up to +-127 are exact in
bf16, ternary is exact in fp8e4, and PSUM accumulates in f32, so every matmul
is an exact integer matmul; per-token / per-tensor dequant scales are applied
to the f32 partial sums afterward.

Sharding (per core r of 8):
  w_gate/w_up: I-column shard (1024 of 8192)  -> h^T shard [I_sh=1024, T]
  w_down:      I-row shard                    -> partial y, ReduceScatter(add)
  per-token RMS var and abs-max stats over the full I: AllGather of stats.

Quantized activations are stored as RAW integers (bf16); the full per-token
scale mc = max|x|/127 is folded into one post-matmul multiply.  Intermediate
h is kept in bf16 (safe: quant decisions have >2x margin to the 0.5 rounding
boundary).  Elementwise work is split across Vector (DVE) and Scalar (ACT)
engines; x tiles are prefetched a full group ahead so neither the PE nor the
DVE queue ever head-blocks on DMA latency.
"""
import numpy as np

N_CORES = 8
B, S, H, I = 2, 2048, 2048, 8192
T = B * S                      # 4096 tokens
ISH = I // N_CORES             # 1024  I shard per core
TG = 512                       # tokens per group
NG = T // TG                   # 8 groups
KC = H // 128                  # 16 contract chunks for gate/up
IC = ISH // 128                # 8  contract chunks for down / h^T partition chunks
NH = 2048 // 512               # 4  output col groups for down
NTC = TG // 128                # 4  token tiles per group
RPB = TG // N_CORES            # 64 output rows per group per core

MAGIC = float(1.5 * 2 ** 23)   # f32 round-to-nearest-even forcing constant
EPS = 1e-5
RMS_EPS = 1e-6

_CACHED = {}


def _build():
    import concourse.bass as bass
    import concourse.bacc as bacc
    import concourse.tile as tile
    import concourse.mybir as mybir
    from concourse import masks
    from contextlib import ExitStack

    dt = mybir.dt
    AO = mybir.AluOpType
    AF = mybir.ActivationFunctionType
    AX = mybir.AxisListType
    RG = [list(range(N_CORES))]

    nc = bacc.Bacc("TRN2", target_bir_lowering=False, debug=False,
                   num_devices=N_CORES)

    xT_in = nc.dram_tensor("xT", [H, T], dt.float32, kind="ExternalInput")
    wgT_in = nc.dram_tensor("wgT", [H, ISH], dt.float32, kind="ExternalInput")
    wuT_in = nc.dram_tensor("wuT", [H, ISH], dt.float32, kind="ExternalInput")
    wdT_in = nc.dram_tensor("wdT", [ISH, 2048], dt.float32, kind="ExternalInput")
    lnw_in = nc.dram_tensor("lnw", [ISH], dt.float32, kind="ExternalInput")
    y_out = nc.dram_tensor("y_out", [T // N_CORES, 2048], dt.bfloat16,
                           kind="ExternalOutput")

    with tile.TileContext(nc) as tc:
        with ExitStack() as stack:
            ep = stack.enter_context
            constp = ep(tc.tile_pool(name="const", bufs=1))
            wqp = ep(tc.tile_pool(name="wq", bufs=1))
            wstage = ep(tc.tile_pool(name="wstage", bufs=3))
            xsp = ep(tc.tile_pool(name="xs", bufs=7))
            xqp = ep(tc.tile_pool(name="xq", bufs=5))
            stp = ep(tc.tile_pool(name="st32", bufs=1))
            xmp = ep(tc.tile_pool(name="xm", bufs=2))
            qxp = ep(tc.tile_pool(name="qx", bufs=2))
            hbp = ep(tc.tile_pool(name="hbuf", bufs=2))
            qhp = ep(tc.tile_pool(name="qh", bufs=2))
            bcp = ep(tc.tile_pool(name="bc", bufs=2))
            evp = ep(tc.tile_pool(name="evac", bufs=3))
            evq = ep(tc.tile_pool(name="evq", bufs=2))
            mxp = ep(tc.tile_pool(name="mx", bufs=2))
            smp = ep(tc.tile_pool(name="small", bufs=2))
            rowp = ep(tc.tile_pool(name="rows", bufs=2))
            yrp = ep(tc.tile_pool(name="yrow", bufs=2))
            ps_gu = ep(tc.tile_pool(name="ps_gu", bufs=3, space="PSUM"))
            ps_dn = ep(tc.tile_pool(name="ps_dn", bufs=2, space="PSUM"))
            ps_ss = ep(tc.tile_pool(name="ps_ss", bufs=1, space="PSUM"))
            ps_misc = ep(tc.tile_pool(name="ps_misc", bufs=2, space="PSUM"))
            dram = ep(tc.tile_pool(name="dram", bufs=1, space="DRAM"))
            dram_rs = ep(tc.tile_pool(name="dram_rs", bufs=8, space="DRAM"))

            # ---------- constants ----------
            ident = constp.tile([128, 128], dt.float32)
            masks.make_identity(nc, ident[:])
            ident_bf = constp.tile([128, 128], dt.bfloat16)
            nc.vector.tensor_copy(ident_bf[:], ident[:])
            ones_col = constp.tile([128, 1], dt.float32)
            nc.vector.memset(ones_col[:], 1.0)
            ones_col_bf = constp.tile([128, 1], dt.bfloat16)
            nc.vector.memset(ones_col_bf[:], 1.0)
            ones_row = constp.tile([1, 128], dt.float32)
            nc.vector.memset(ones_row[:], 1.0)
            lnw_sb = constp.tile([128, IC], dt.float32)    # lnw[128*ic + p] at [p, ic]
            nc.sync.dma_start(lnw_sb[:], lnw_in.rearrange("(c p) -> p c", p=128)[:])

            # ---------- internal DRAM ----------
            y_partial = dram.tile([T, 2048], dt.bfloat16)
            stat_in = dram.tile([NG, 2, TG], dt.float32)
            stat_out = dram.tile([NG, 2 * N_CORES, TG], dt.float32)
            wsum_part = dram.tile([8], dt.float32)
            wsum_glob = dram.tile([8], dt.float32)
            row_bounce = dram.tile([NG, 4, TG], dt.float32)  # sx / mc / al / cd

            W_SPECS = ((wgT_in, KC, ISH), (wuT_in, KC, ISH), (wdT_in, IC, 2048))

            # ---------- slot registries for the software pipeline ----------
            xmax_slots = {}
            sx_slots = {}
            mc_slots = {}
            al_slots = {}
            cd_slots = {}
            qxT_slots = {}
            hT_slots = {}
            qh_slots = {}
            rs_slots = {}

            U32 = dt.uint32

            def emit_pre_loads(g):
                tok0 = g * TG
                tiles = []
                for kc in range(KC):
                    st = xsp.tile([128, TG], dt.float32, tag="xs")
                    nc.sync.dma_start(st[:], xT_in[kc * 128:(kc + 1) * 128,
                                                   tok0:tok0 + TG])
                    tiles.append(st)
                return tiles

            def emit_absmax(g, tiles):
                xmax = xmp.tile([128, TG], dt.float32, tag="xmax")
                xmax_slots[g] = xmax
                for kc, st in enumerate(tiles):
                    if kc == 0:
                        nc.scalar.activation(xmax[:], st[:], AF.Abs)
                    else:
                        nc.scalar.activation(st[:], st[:], AF.Abs)
                        nc.vector.tensor_tensor(xmax[:], xmax[:], st[:], AO.max)

            def emit_mid(g):
                # per-token absmax -> sx (=127/mx) and mc (=mx/127), bounce to
                # DRAM and broadcast back across partitions
                xmax = xmax_slots.pop(g)
                mx = smp.tile([128, NTC], dt.float32, tag="mx_nat")
                for c in range(NTC):
                    tr = ps_misc.tile([128, 512], dt.float32, tag="misc_ps")
                    nc.tensor.transpose(tr[:, 0:128],
                                        xmax[:, c * 128:(c + 1) * 128], ident[:])
                    nc.vector.tensor_reduce(mx[:, c:c + 1], tr[:, 0:128],
                                            AX.X, AO.max)
                nc.vector.tensor_scalar(mx[:], mx[:], EPS, None, AO.max)
                sm = smp.tile([128, 2 * NTC], dt.float32, tag="sm")
                nc.vector.tensor_scalar(sm[:, NTC:2 * NTC], mx[:],
                                        float(1.0 / 127.0), None, AO.mult)
                r0 = smp.tile([128, NTC], dt.float32, tag="sx_r0")
                nc.vector.reciprocal(r0[:], mx[:])
                ntr = smp.tile([128, NTC], dt.float32, tag="sx_nt")
                nc.vector.tensor_tensor(ntr[:], mx[:], r0[:], AO.mult)
                nc.vector.tensor_scalar(ntr[:], ntr[:], -1.0, 2.0, AO.mult, AO.add)
                nc.vector.tensor_tensor(sm[:, 0:NTC], r0[:], ntr[:], AO.mult)
                nc.vector.tensor_scalar(sm[:, 0:NTC], sm[:, 0:NTC], 127.0, None,
                                        AO.mult)
                nc.sync.dma_start(
                    row_bounce[g, 0:2].rearrange("s (c p) -> p s c", p=128)[:],
                    sm.rearrange("p (s c) -> p s c", c=NTC)[:])
                sx_tile = bcp.tile([128, TG], dt.float32, tag="sx_tile")
                sx_slots[g] = sx_tile
                nc.sync.dma_start(sx_tile[:], row_bounce[g, 0]
                                  .rearrange("(o f) -> o f", o=1)
                                  .partition_broadcast(128))
                mc_tile = bcp.tile([128, TG], dt.float32, tag="mc_tile")
                mc_slots[g] = mc_tile
                nc.sync.dma_start(mc_tile[:], row_bounce[g, 1]
                                  .rearrange("(o f) -> o f", o=1)
                                  .partition_broadcast(128))

            def emit_qnt(g):
                # re-read x, quantize to raw ints in bf16
                tok0 = g * TG
                sx_tile = sx_slots.pop(g)
                qxT = qxp.tile([128, KC * TG], dt.bfloat16, tag="qxT")
                qxT_slots[g] = qxT
                for kc in range(KC):
                    st = xqp.tile([128, TG], dt.float32, tag="xq")
                    nc.sync.dma_start(st[:], xT_in[kc * 128:(kc + 1) * 128,
                                                   tok0:tok0 + TG])
                    nc.vector.tensor_tensor(st[:], st[:], sx_tile[:], AO.mult)
                    nc.vector.tensor_scalar(qxT[:, kc * TG:(kc + 1) * TG], st[:],
                                            MAGIC, -MAGIC, AO.add, AO.add)

            def emit_phase1(g):
                qxT = qxT_slots.pop(g)
                mc_tile = mc_slots.pop(g)
                hT = hbp.tile([128, IC * TG], dt.bfloat16, tag="hT")
                hT_slots[g] = hT
                maxt = mxp.tile([128, TG], dt.float32, tag="maxt")
                ss_ps = ps_ss.tile([1, TG], dt.float32, tag="ss_ps")
                h2_prev = [None]

                def emit_ss(ic_done):
                    nc.tensor.matmul(ss_ps[:], ones_col_bf[:], h2_prev[0][:],
                                     start=(ic_done == 0), stop=(ic_done == IC - 1))

                for ic in range(IC):
                    g_ps = ps_gu.tile([128, TG], dt.float32, tag="gu_ps")
                    u_ps = ps_gu.tile([128, TG], dt.float32, tag="gu_ps")
                    for kc in range(KC):
                        nc.tensor.matmul(
                            g_ps[:],
                            qwg[:, kc * ISH + ic * 128: kc * ISH + (ic + 1) * 128],
                            qxT[:, kc * TG:(kc + 1) * TG],
                            start=(kc == 0), stop=(kc == KC - 1))
                    for kc in range(KC):
                        nc.tensor.matmul(
                            u_ps[:],
                            qwu[:, kc * ISH + ic * 128: kc * ISH + (ic + 1) * 128],
                            qxT[:, kc * TG:(kc + 1) * TG],
                            start=(kc == 0), stop=(kc == KC - 1))
                    if ic > 0:
                        emit_ss(ic - 1)
                    gv = evp.tile([128, TG], dt.float32, tag="gv")
                    nc.vector.tensor_tensor(gv[:], g_ps[:], mc_tile[:], AO.mult)
                    nc.scalar.activation(gv[:], gv[:], AF.Silu,
                                         scale=wstats[:, 3:4])
                    hsl = hT[:, ic * TG:(ic + 1) * TG]
                    nc.vector.scalar_tensor_tensor(hsl, u_ps[:], wstats[:, 4:5],
                                                   gv[:], AO.mult, AO.mult)
                    h2 = evq.tile([128, TG], dt.bfloat16, tag="h2")
                    nc.scalar.square(h2[:], hsl)
                    h2_prev[0] = h2
                    if ic == 0:
                        nc.scalar.activation(maxt[:], hsl, AF.Abs,
                                             scale=lnw_sb[:, 0:1])
                    else:
                        ha = evq.tile([128, TG], dt.bfloat16, tag="ha")
                        nc.scalar.activation(ha[:], hsl, AF.Abs,
                                             scale=lnw_sb[:, ic:ic + 1])
                        nc.vector.tensor_tensor(maxt[:], maxt[:], ha[:], AO.max)
                emit_ss(IC - 1)
                pm_nat = smp.tile([128, NTC], dt.float32, tag="pm_nat")
                for c in range(NTC):
                    tr = ps_misc.tile([128, 512], dt.float32, tag="misc_ps")
                    nc.tensor.transpose(tr[:, 0:128],
                                        maxt[:, c * 128:(c + 1) * 128],
                                        ident[:])
                    nc.vector.tensor_reduce(pm_nat[:, c:c + 1], tr[:, 0:128],
                                            AX.X, AO.max)
                ss_row = stp.tile([1, TG], dt.float32, tag="ss_row")
                nc.vector.tensor_copy(ss_row[:], ss_ps[:])
                nc.gpsimd.dma_start(stat_in[g, 0].rearrange("(o f) -> o f", o=1)[:],
                                    ss_row[:])
                nc.gpsimd.dma_start(stat_in[g, 1].rearrange("(c p) -> p c", p=128)[:],
                                    pm_nat[:])
                nc.gpsimd.collective_compute(
                    "AllGather", AO.bypass, replica_groups=RG,
                    ins=[stat_in[g].opt()], outs=[stat_out[g].opt()])

            def emit_phase2a(g):
                J = TG // 32
                # gathered stats [16, TG] -> [32, TG] tile; rows 16:32 zeroed
                stat32 = stp.tile([32, TG], dt.float32, tag="stat32")
                nc.vector.memset(stat32[:], 0.0)
                nc.gpsimd.dma_start(stat32[0:2 * N_CORES, :], stat_out[g])
                st32 = stp.tile([32, TG], dt.float32, tag="st32")
                nc.vector.transpose(st32[:], stat32[:])
                # st32[q, 32j + 16h + 2a + kind]: token t=32j+q, rank a, h=1 junk
                stv = st32.rearrange("p (j h a two) -> p j h two a",
                                     h=2, two=2, a=N_CORES)
                ssg = smp.tile([32, J], dt.float32, tag="ssg")
                nc.vector.tensor_reduce(ssg[:], stv[:, :, 0:1, 0:1, :],
                                        AX.X, AO.add)
                pmg = smp.tile([32, J], dt.float32, tag="pmg")
                nc.vector.tensor_reduce(pmg[:], stv[:, :, 0:1, 1:2, :],
                                        AX.X, AO.max)
                mc32 = smp.tile([32, J], dt.float32, tag="mc32")
                nc.sync.dma_start(mc32[:], row_bounce[g, 1]
                                  .rearrange("(j q) -> q j", q=32)[:])
                nc.vector.tensor_tensor(pmg[:], pmg[:], mc32[:], AO.mult)
                rr2 = smp.tile([32, J], dt.float32, tag="rr2")
                nc.vector.tensor_tensor(rr2[:], mc32[:], mc32[:], AO.mult)
                nc.vector.tensor_tensor(ssg[:], ssg[:], rr2[:], AO.mult)
                vr = smp.tile([32, J], dt.float32, tag="vr")
                nc.vector.tensor_scalar(vr[:], ssg[:], float(1.0 / I), RMS_EPS,
                                        AO.mult, AO.add)
                sq = smp.tile([32, J], dt.float32, tag="sq")
                nc.scalar.sqrt(sq[:], vr[:])
                rr = smp.tile([32, J], dt.float32, tag="rr")
                nc.vector.reciprocal(rr[:], sq[:])
                ntn = smp.tile([32, J], dt.float32, tag="ntn")
                nc.vector.tensor_tensor(ntn[:], sq[:], rr[:], AO.mult)
                nc.vector.tensor_scalar(ntn[:], ntn[:], -1.0, 2.0, AO.mult, AO.add)
                nc.vector.tensor_tensor(rr[:], rr[:], ntn[:], AO.mult)
                rmc = smp.tile([32, J], dt.float32, tag="rmc")
                nc.vector.tensor_tensor(rmc[:], rr[:], pmg[:], AO.mult)
                nc.vector.tensor_scalar(rmc[:], rmc[:], EPS, None, AO.max)
                cd32 = smp.tile([32, J], dt.float32, tag="cd32")
                nc.vector.tensor_scalar(cd32[:], rmc[:], wstats[0:32, 5:6],
                                        float(1.0 / 127.0), AO.mult, AO.mult)
                nc.sync.dma_start(row_bounce[g, 3]
                                  .rearrange("(j q) -> q j", q=32)[:], cd32[:])
                cd = smp.tile([128, NTC], dt.float32, tag="cd")
                cd_slots[g] = cd
                nc.sync.dma_start(cd[:], row_bounce[g, 3]
                                  .rearrange("(c p) -> p c", p=128)[:])
                ar0 = smp.tile([32, J], dt.float32, tag="ar0")
                nc.vector.reciprocal(ar0[:], rmc[:])
                ntn2 = smp.tile([32, J], dt.float32, tag="ntn2")
                nc.vector.tensor_tensor(ntn2[:], rmc[:], ar0[:], AO.mult)
                nc.vector.tensor_scalar(ntn2[:], ntn2[:], -1.0, 2.0, AO.mult, AO.add)
                nc.vector.tensor_tensor(ar0[:], ar0[:], ntn2[:], AO.mult)
                al32 = smp.tile([32, J], dt.float32, tag="al32")
                nc.vector.tensor_tensor(al32[:], rr[:], ar0[:], AO.mult)
                nc.vector.tensor_scalar(al32[:], al32[:], 127.0, None, AO.mult)
                nc.vector.tensor_tensor(al32[:], al32[:], mc32[:], AO.mult)
                nc.sync.dma_start(row_bounce[g, 2]
                                  .rearrange("(j q) -> q j", q=32)[:], al32[:])
                al_tile = bcp.tile([128, TG], dt.float32, tag="al_tile")
                al_slots[g] = al_tile
                nc.sync.dma_start(al_tile[:], row_bounce[g, 2]
                                  .rearrange("(o f) -> o f", o=1)
                                  .partition_broadcast(128))

            def emit_phase2q(g):
                hT = hT_slots.pop(g)
                al_tile = al_slots.pop(g)
                qhT = qhp.tile([128, IC * TG], dt.bfloat16, tag="qhT")
                qh_slots[g] = qhT
                for ic in range(IC):
                    tq = evp.tile([128, TG], dt.float32, tag="tq")
                    nc.vector.scalar_tensor_tensor(tq[:], hT[:, ic * TG:(ic + 1) * TG],
                                                   lnw_sb[:, ic:ic + 1], al_tile[:],
                                                   AO.mult, AO.mult)
                    nc.vector.tensor_scalar(qhT[:, ic * TG:(ic + 1) * TG], tq[:],
                                            MAGIC, -MAGIC, AO.add, AO.add)

            def emit_phase2d(g):
                tok0 = g * TG
                qhT = qh_slots.pop(g)
                cd = cd_slots.pop(g)
                for tcx in range(NTC):
                    y_row = yrp.tile([128, 2048], dt.bfloat16, tag="y_row")
                    for nh in range(NH):
                        y_ps = ps_dn.tile([128, 512], dt.float32, tag="y_ps")
                        for ic in range(IC):
                            nc.tensor.matmul(
                                y_ps[:],
                                qhT[:, ic * TG + tcx * 128: ic * TG + (tcx + 1) * 128],
                                qwd[:, ic * 2048 + nh * 512: ic * 2048 + (nh + 1) * 512],
                                start=(ic == 0), stop=(ic == IC - 1))
                        nc.scalar.mul(y_row[:, nh * 512:(nh + 1) * 512], y_ps[:],
                                      cd[:, tcx:tcx + 1])
                    nc.sync.dma_start(
                        y_partial[tok0 + tcx * 128: tok0 + (tcx + 1) * 128, :],
                        y_row[:])
                rs_out = dram_rs.tile([RPB, 2048], dt.bfloat16, tag="rs_out")
                rs_slots[g] = rs_out
                nc.gpsimd.collective_compute(
                    "ReduceScatter", AO.add, replica_groups=RG,
                    ins=[y_partial[tok0:tok0 + TG, :].opt()],
                    outs=[rs_out.opt()])

            def emit_ycast(g):
                nc.sync.dma_start(y_out[g * RPB:(g + 1) * RPB, :],
                                  rs_slots.pop(g)[:])

            # ================= preamble =================
            pre_tiles = {0: emit_pre_loads(0)}

            # weight abs-sum pass: ACT abs + free-axis accumulate, PE reduce
            wsum_row = rowp.tile([1, 8], dt.float32, tag="wsum_row")
            for wi, (w_in, nchunk, wcols) in enumerate(W_SPECS):
                acc = constp.tile([128, 1], dt.float32, tag=f"wacc{wi}")
                first = True
                for c in range(nchunk):
                    for cc in range(wcols // 1024):
                        st = wstage.tile([128, 1024], dt.float32, tag="wstage")
                        nc.sync.dma_start(st[:], w_in[c * 128:(c + 1) * 128,
                                                      cc * 1024:(cc + 1) * 1024])
                        red = smp.tile([128, 1], dt.float32, tag="wred")
                        nc.scalar.activation(st[:], st[:], AF.Abs,
                                             accum_out=red[:])
                        if first:
                            nc.vector.tensor_copy(acc[:], red[:])
                            first = False
                        else:
                            nc.vector.tensor_tensor(acc[:], acc[:], red[:], AO.add)
                wsum_ps = ps_misc.tile([128, 512], dt.float32, tag="misc_ps")
                nc.tensor.matmul(wsum_ps[0:1, 0:1], ones_col[:], acc[:],
                                 start=True, stop=True)
                nc.scalar.copy(wsum_row[:, wi:wi + 1], wsum_ps[0:1, 0:1])
            nc.vector.memset(wsum_row[:, 3:8], 0.0)

            pre_tiles[1] = emit_pre_loads(1)
            emit_absmax(0, pre_tiles.pop(0))
            emit_mid(0)
            emit_qnt(0)

            nc.sync.dma_start(wsum_part.rearrange("(o f) -> o f", o=1)[:],
                              wsum_row[:])
            nc.gpsimd.collective_compute(
                "AllReduce", AO.add, replica_groups=RG,
                ins=[wsum_part.opt()], outs=[wsum_glob.opt()])

            # scl_row: [sw_g, sw_u, sw_d, mean_g, mean_u, mean_d, 0, 0]
            wsg_row = rowp.tile([1, 8], dt.float32, tag="wsg_row")
            nc.sync.dma_start(wsg_row[:], wsum_glob.rearrange("(o f) -> o f", o=1)[:])
            mean_row = rowp.tile([1, 8], dt.float32, tag="mean_row")
            nc.vector.tensor_scalar(mean_row[:, 0:3], wsg_row[:, 0:3],
                                    float(1.0 / (I * H)), EPS, AO.mult, AO.max)
            scl_row = rowp.tile([1, 8], dt.float32, tag="scl_row")
            rw = rowp.tile([1, 8], dt.float32, tag="rw_row")
            nc.vector.reciprocal(rw[:, 0:3], mean_row[:, 0:3])
            nt = rowp.tile([1, 8], dt.float32, tag="nt_row")
            nc.vector.tensor_tensor(nt[:, 0:3], mean_row[:, 0:3], rw[:, 0:3], AO.mult)
            nc.vector.tensor_scalar(nt[:, 0:3], nt[:, 0:3], -1.0, 2.0, AO.mult, AO.add)
            nc.vector.tensor_tensor(scl_row[:, 0:3], rw[:, 0:3], nt[:, 0:3], AO.mult)
            nc.vector.tensor_copy(scl_row[:, 3:5], mean_row[:, 0:2])
            nc.vector.tensor_copy(scl_row[:, 5:6], mean_row[:, 2:3])
            nc.vector.memset(scl_row[:, 6:8], 0.0)
            wst_ps = ps_misc.tile([128, 512], dt.float32, tag="misc_ps")
            nc.tensor.matmul(wst_ps[:, 0:8], ones_row[:], scl_row[:], start=True,
                             stop=True)
            wstats = constp.tile([128, 8], dt.float32)
            nc.vector.tensor_copy(wstats[:], wst_ps[:, 0:8])

            # quantize weights to ternary fp8 (ACT scale + DVE clamp/round)
            qwg = wqp.tile([128, KC * ISH], dt.float8e4)
            qwu = wqp.tile([128, KC * ISH], dt.float8e4)
            qwd = wqp.tile([128, IC * 2048], dt.float8e4)
            for (w_in, qw, nchunk, wcols, si) in (
                (wgT_in, qwg, KC, ISH, 0), (wuT_in, qwu, KC, ISH, 1),
                (wdT_in, qwd, IC, 2048, 2),
            ):
                for c in range(nchunk):
                    for cc in range(wcols // 1024):
                        st = wstage.tile([128, 1024], dt.float32, tag="wstage")
                        nc.sync.dma_start(st[:], w_in[c * 128:(c + 1) * 128,
                                                      cc * 1024:(cc + 1) * 1024])
                        nc.scalar.activation(st[:], st[:], AF.Copy,
                                             scale=wstats[:, si:si + 1])
                        nc.vector.tensor_scalar(st[:], st[:], 1.0, -1.0,
                                                AO.min, AO.max)
                        o0 = c * wcols + cc * 1024
                        nc.vector.tensor_scalar(qw[:, o0:o0 + 1024], st[:],
                                                MAGIC, -MAGIC, AO.add, AO.add)

            emit_absmax(1, pre_tiles.pop(1))
            emit_mid(1)
            emit_qnt(1)
            pre_tiles[2] = emit_pre_loads(2)

            # ================= main pipeline =================
            for g in range(NG):
                if g + 3 < NG:
                    pre_tiles[g + 3] = emit_pre_loads(g + 3)
                if g + 2 < NG:
                    emit_absmax(g + 2, pre_tiles.pop(g + 2))
                emit_phase1(g)
                if g >= 1:
                    emit_phase2a(g - 1)
                    emit_phase2q(g - 1)
                if g + 2 < NG:
                    emit_mid(g + 2)
                    emit_qnt(g + 2)
                if g >= 2:
                    emit_phase2d(g - 2)
                if g >= 4:
                    emit_ycast(g - 4)
            emit_phase2d(NG - 2)
            emit_phase2a(NG - 1)
            emit_phase2q(NG - 1)
            emit_phase2d(NG - 1)
            for g in range(NG - 4, NG):
                emit_ycast(g)

    nc.compile()
    return nc


def _get_nc():
    if "nc" not in _CACHED:
        _CACHED["nc"] = _build()
    return _CACHED["nc"]


def _make_in_maps(x, w_gate, w_up, w_down, ln_weight):
    xf = np.ascontiguousarray(np.asarray(x, dtype=np.float32).reshape(T, H).T)
    wgT = np.asarray(w_gate, dtype=np.float32).T   # [H, I]
    wuT = np.asarray(w_up, dtype=np.float32).T     # [H, I]
    wdT = np.asarray(w_down, dtype=np.float32).T   # [I, H]
    lnw = np.asarray(ln_weight, dtype=np.float32)
    in_maps = []
    for r in range(N_CORES):
        c0 = r * ISH
        in_maps.append({
            "xT": xf,
            "wgT": np.ascontiguousarray(wgT[:, c0:c0 + ISH]),
            "wuT": np.ascontiguousarray(wuT[:, c0:c0 + ISH]),
            "wdT": np.ascontiguousarray(wdT[c0:c0 + ISH, :]),
            "lnw": np.ascontiguousarray(lnw[c0:c0 + ISH]),
        })
    return in_maps


def _assemble(results):
    out = np.empty((T, 2048), dtype=np.float32)
    for r in range(N_CORES):
        yr = np.asarray(results[r]["y_out"]).astype(np.float32)
        for g in range(NG):
            t0 = g * TG + r * RPB
            out[t0:t0 + RPB] = yr[g * RPB:(g + 1) * RPB]
    return out.reshape(B, S, 2048)


def kernel(x, w_gate, w_up, w_down, ln_weight):
    from concourse import bass_utils

    nc = _get_nc()
    in_maps = _make_in_maps(x, w_gate, w_up, w_down, ln_weight)
    res = bass_utils.run_bass_kernel_spmd(nc, in_maps,
                                          core_ids=list(range(N_CORES)))
    return _assemble(res.results)
